# revision 1
# baseline (speedup 1.0000x reference)
"""Trainium2 Bass kernel for nn_LCNNConvolution (GNN message passing).

Math:  out[n] = sum_p softplus( gather(X, NS[n,p,:]).flat @ W.T + b ) - 12*ln2
Key transform: W is block-structured over the 8 neighbor slots, so
    x1[n,p,:] = sum_k Y_k[NS[n,p,k]]        with  Y_k = X @ W_k.T  (+ b/1 baked
into slot 7). We precompute Y on-chip (PE matmul, fp16), write it to DRAM as
[site, 8*64] rows, then the hot loop is an indirect-DMA gather of 128B rows +
DVE reduction over the 8 slots + ACT softplus + DVE reduction over 12 perms.

Sharding: data-parallel over sites; each of the 8 cores handles 6250 sites and
computes its own full Y copy (replicated X / W).
"""

import math
import os

import numpy as np

import concourse.bass as bass
import concourse.bacc as bacc
import concourse.mybir as mybir
import concourse.tile as tile
from concourse.bass_utils import run_bass_kernel_spmd

# ---------------------------------------------------------------- constants
N_SITES = 50000
NODE_F = 64
N_PERM = 12
N_NEIGH = 8
OUT_F = 64
LN2 = float(np.log(2.0))

N_CORES = 8
SITES_PER_CORE = N_SITES // N_CORES            # 6250
SITES_PER_PART = 50                            # ceil(6250/128) padded to 50
PAD_SITES = 128 * SITES_PER_PART               # 6400
COLS = SITES_PER_PART * N_PERM                 # 600 rows (n,p) per partition
GCOLS = 8                                      # cols per dma_gather call
N_CHUNKS = COLS // GCOLS                       # 75 gather chunks
NIDX = 128 * GCOLS                             # 1024 gathers/call (HW limit)
RCOLS = 24                                     # cols per reduce group (2 sites)
BANK = 32767                                   # bank A covers sites [0, 32767)
DUMMY_B = 50001 - BANK                         # zero row for bank B

XT_HALF = 25088                                # 196*128, top half site count
YROWS = N_SITES                                # Y table rows

F32 = mybir.dt.float32
F16 = mybir.dt.float16
I32 = mybir.dt.int32

I16 = mybir.dt.int16
Y_DT = F32  # dma_gather needs 256B elements -> 64 x f32 rows


# ---------------------------------------------------------------- device IR
def build_nc(y_dt=Y_DT):
    nc = bacc.Bacc("TRN2", target_bir_lowering=False, debug=False)

    xt = nc.dram_tensor("xt", [128, XT_HALF], F32, kind="ExternalInput").ap()
    wt = nc.dram_tensor("wt", [128, 512], F32, kind="ExternalInput").ap()
    bz = nc.dram_tensor("bz", [1, 64], F32, kind="ExternalInput").ap()
    # per chunk: 16 (slot, bank) index sets, 16-partition-wrapped + replicated
    idx = nc.dram_tensor(
        "idx", [N_CHUNKS, 128, 16 * (NIDX // 16)], I16, kind="ExternalInput"
    ).ap()
    out = nc.dram_tensor(
        "out", [128, SITES_PER_PART, OUT_F], F32, kind="ExternalOutput"
    ).ap()

    with tile.TileContext(nc) as tc:
        with (
            tc.tile_pool(name="persist", bufs=1) as persist,
            tc.tile_pool(name="dram", bufs=1, space="DRAM") as dram,
        ):
            half_sb = persist.tile([128, 1], F32)
            nc.vector.memset(half_sb[:], 0.5)

            # rows: [Z, site 0..49999, Z2] — zero rows are the dummy targets
            ybig = dram.tile([YROWS + 2, 512], y_dt)
            zrow = persist.tile([1, 512], F32)
            nc.vector.memset(zrow[:], 0.0)
            nc.sync.dma_start(out=ybig[0:1, :], in_=zrow[:])
            nc.sync.dma_start(out=ybig[YROWS + 1 : YROWS + 2, :], in_=zrow[:])

            # ---------------- phase 1: Y = X @ Wall.T  (+bias in slot 7)
            with (
                tc.tile_pool(name="p1", bufs=1) as p1,
                tc.tile_pool(name="p1y", bufs=4) as p1y,
                tc.tile_pool(name="p1ps", bufs=4, space="PSUM") as p1ps,
            ):
                xt_sb = p1.tile([128, XT_HALF], F32)
                nc.sync.dma_start(out=xt_sb[:], in_=xt[:])
                wt_sb = p1.tile([128, 512], F32)
                nc.sync.dma_start(out=wt_sb[:], in_=wt[:])
                bz_sb = p1.tile([1, 64], F32)
                nc.sync.dma_start(out=bz_sb[:], in_=bz[:])
                ones_sb = p1.tile([1, 128], F32)
                nc.vector.memset(ones_sb[:], 1.0)

                for h in range(2):
                    for j in range(XT_HALF // 128):
                        s0 = h * XT_HALF + j * 128  # first site of this block
                        if s0 >= N_SITES:
                            break
                        nrows = min(128, N_SITES - s0)
                        psum = p1ps.tile([128, 512], F32, space="PSUM", tag="ps")
                        lhsT = xt_sb[64 * h : 64 * h + 64, j * 128 : (j + 1) * 128]
                        nc.tensor.matmul(
                            out=psum[:, 0:448],
                            lhsT=lhsT,
                            rhs=wt_sb[64 * h : 64 * h + 64, 0:448],
                            start=True,
                            stop=True,
                        )
                        nc.tensor.matmul(
                            out=psum[:, 448:512],
                            lhsT=lhsT,
                            rhs=wt_sb[64 * h : 64 * h + 64, 448:512],
                            start=True,
                            stop=False,
                        )
                        nc.tensor.matmul(
                            out=psum[:, 448:512],
                            lhsT=ones_sb[:1, :128],
                            rhs=bz_sb[:1, :64],
                            start=False,
                            stop=True,
                        )
                        y_sb = p1y.tile([128, 512], y_dt, tag="y")
                        nc.scalar.copy(out=y_sb[:], in_=psum[:])
                        nc.sync.dma_start(
                            out=ybig[1 + s0 : 1 + s0 + nrows, :], in_=y_sb[:nrows, :]
                        )

            # ---------------- phase 2: dma_gather (2 banks x 8 slots) + reduce
            # For each output row r and slot k: site s gathered from ybig rows
            # [1+s] via bank A (idx=s+1, dummy 0 -> zero row) or bank B
            # (idx=s-BANK+1, dummy DUMMY_B -> zero row). x1 = sum of all 16.
            with (
                tc.tile_pool(name="p2g", bufs=2) as p2g,
                tc.tile_pool(name="p2i", bufs=2) as p2i,
                tc.tile_pool(name="p2o", bufs=2) as p2o,
            ):
                x1 = None
                for j in range(N_CHUNKS):
                    idx_sb = p2i.tile([128, 16 * (NIDX // 16)], I16, tag="idx")
                    nc.sync.dma_start(out=idx_sb[:], in_=idx[j])
                    g = p2g.tile([128, 16, GCOLS, OUT_F], F32, tag="g")
                    for kb in range(16):
                        k, bank = kb // 2, kb % 2
                        if bank == 0:
                            tab = ybig[0:BANK, k * 64 : (k + 1) * 64]
                        else:
                            tab = ybig[BANK : YROWS + 2, k * 64 : (k + 1) * 64]
                        nc.gpsimd.dma_gather(
                            out_ap=g[:, kb, :, :],
                            in_ap=tab,
                            idxs_ap=idx_sb[
                                :, kb * (NIDX // 16) : (kb + 1) * (NIDX // 16)
                            ],
                            num_idxs=NIDX,
                            num_idxs_reg=NIDX,
                            elem_size=64,
                            elem_step=512,
                        )
                    # x1[p, c, f] = sum_kb g[p, kb, c, f] into its 8-col slice
                    if j % 3 == 0:
                        x1 = p2o.tile([128, RCOLS, OUT_F], F32, tag="x1")
                    sub = j % 3
                    nc.vector.tensor_reduce(
                        out=x1[:, sub * GCOLS : (sub + 1) * GCOLS, :],
                        in_=g[:].rearrange("p k c f -> p c f k"),
                        axis=mybir.AxisListType.X,
                        op=mybir.AluOpType.add,
                    )
                    if sub != 2:
                        continue
                    grp = j // 3  # 24-col group = 2 sites
                    # softplus(x) - ln2 == Ln(0.5*Exp(x) + 0.5)
                    x2 = p2o.tile([128, RCOLS, OUT_F], F32, tag="x2")
                    nc.scalar.activation(
                        out=x2[:],
                        in_=x1[:],
                        func=mybir.ActivationFunctionType.Exp,
                    )
                    nc.scalar.activation(
                        out=x2[:],
                        in_=x2[:],
                        func=mybir.ActivationFunctionType.Ln,
                        scale=0.5,
                        bias=half_sb[:],
                    )
                    # out[p, s, f] = sum_q x2[p, s*12+q, f]
                    acc = p2o.tile([128, RCOLS // N_PERM, OUT_F], F32, tag="acc")
                    nc.vector.tensor_reduce(
                        out=acc[:],
                        in_=x2[:].rearrange("p (s q) f -> p s f q", q=N_PERM),
                        axis=mybir.AxisListType.X,
                        op=mybir.AluOpType.add,
                    )
                    nc.sync.dma_start(
                        out=out[:, grp * 2 : grp * 2 + 2, :],
                        in_=acc[:],
                    )

    nc.compile()
    return nc


# ---------------------------------------------------------------- host side
def _host_prep(X_sites, X_NSs, W, b):
    X_sites = np.asarray(X_sites, dtype=np.float32)
    X_NSs = np.asarray(X_NSs)
    W = np.asarray(W, dtype=np.float32)
    b = np.asarray(b, dtype=np.float32)

    xt = np.zeros((128, XT_HALF), dtype=np.float32)
    xt[:64, :] = X_sites[:XT_HALF].T
    xt[64:, : N_SITES - XT_HALF] = X_sites[XT_HALF:].T

    wt = np.ascontiguousarray(
        np.tile(
            W.reshape(OUT_F, N_NEIGH, NODE_F).transpose(2, 1, 0).reshape(NODE_F, 512),
            (2, 1),
        )
    )
    bz = np.ascontiguousarray(b.reshape(1, OUT_F))

    in_maps = []
    for c in range(N_CORES):
        sl = X_NSs[c * SITES_PER_CORE : (c + 1) * SITES_PER_CORE]
        sl = np.concatenate(
            [sl, np.zeros((PAD_SITES - SITES_PER_CORE, N_PERM, N_NEIGH), sl.dtype)]
        )
        s = sl.reshape(128, SITES_PER_PART, N_PERM, N_NEIGH).astype(np.int64)
        # bank A: rows [0, BANK) of ybig -> idx = s+1 (row 1+s), dummy 0 = Z
        a = np.where(s <= BANK - 1, s + 1, 0)
        # bank B: rows [BANK, 50002) -> idx = s-BANK+1, dummy DUMMY_B = Z2
        bk = np.where(s >= BANK, s - BANK + 1, DUMMY_B)
        # V[p, cols, kb] with kb = k*2 + bank, cols = j*12 + q
        V = np.stack([a, bk], axis=-1).reshape(128, COLS, 16)
        # per call (chunk, kb): position i = c*128 + p over 8 cols
        arr = V.reshape(128, N_CHUNKS, GCOLS, 16).transpose(1, 3, 2, 0)
        arr = arr.reshape(N_CHUNKS, 16, NIDX)
        # 16-partition wrap: tile[p_row, col] = arr[col*16 + p_row]
        t16 = arr.reshape(N_CHUNKS, 16, NIDX // 16, 16).transpose(0, 1, 3, 2)
        full = np.tile(
            t16.transpose(0, 2, 1, 3).reshape(N_CHUNKS, 16, NIDX), (1, 8, 1)
        ).astype(np.int16)
        in_maps.append({"xt": xt, "wt": wt, "bz": bz, "idx": full})
    return in_maps


_NC_CACHE = {}


def _get_nc():
    if "nc" not in _NC_CACHE:
        _NC_CACHE["nc"] = build_nc()
    return _NC_CACHE["nc"]


def _stitch(results):
    full = np.empty((N_SITES, OUT_F), dtype=np.float32)
    for c, r in enumerate(results):
        o = r["out"].reshape(PAD_SITES, OUT_F)[:SITES_PER_CORE]
        full[c * SITES_PER_CORE : (c + 1) * SITES_PER_CORE] = o
    return full


def kernel(X_sites, X_NSs, W, b, _trace=False):
    nc = _get_nc()
    in_maps = _host_prep(X_sites, X_NSs, W, b)
    res = run_bass_kernel_spmd(
        nc, in_maps, core_ids=list(range(N_CORES)), trace=_trace
    )
    full = _stitch(res.results)
    if _trace:
        return full, res
    return full



# revision 5
# speedup vs baseline: 2.7495x; 2.7495x over previous
"""v2c: same single-bank pair-table design, but num_idxs=1024 per gather call
(the baseline-proven call size). Gather chunks cover 8 columns; select lands
in a 24-col (2-site) sel buffer; slot-reduce/softplus/perm-reduce fire every
3rd chunk. SPP=50 (pad 6400 sites/core), 75 chunks, 600 gather calls.
"""

import numpy as np

import concourse.bass as bass
import concourse.bacc as bacc
import concourse.mybir as mybir
import concourse.tile as tile
from concourse.bass_utils import run_bass_kernel_spmd

N_SITES = 50000
NODE_F = 64
N_PERM = 12
N_NEIGH = 8
OUT_F = 64

N_CORES = 8
SITES_PER_CORE = N_SITES // N_CORES            # 6250
SPP = 50                                       # sites per partition (pad 6400)
PAD_SITES_CORE = 128 * SPP                     # 6400

SITES_PAD = 50176                              # 2 * 25088 (table pad)
T_ROWS = SITES_PAD // 2                        # 25088 pair rows
NBLK = T_ROWS // 128                           # 196 phase-1 blocks

GCOLS = 8                                      # gather cols per partition/call
NIDX = 128 * GCOLS                             # 1024
RCOLS = 24                                     # reduce group = 2 sites
N_CHUNKS = SPP * N_PERM // GCOLS               # 75

F32 = mybir.dt.float32
F16 = mybir.dt.float16
I16 = mybir.dt.int16
I8 = mybir.dt.int8


def build_nc():
    nc = bacc.Bacc("TRN2", target_bir_lowering=False, debug=False)

    x2t = nc.dram_tensor("x2t", [128, T_ROWS], F16, kind="ExternalInput").ap()
    rhs2 = nc.dram_tensor("rhs2", [128, 1024], F16, kind="ExternalInput").ap()
    bias = nc.dram_tensor("bias", [128, 1024], F32, kind="ExternalInput").ap()
    idx = nc.dram_tensor(
        "idx", [N_CHUNKS, 16, N_NEIGH, NIDX // 16], I16, kind="ExternalInput"
    ).ap()
    msk = nc.dram_tensor(
        "msk", [N_CHUNKS, 128, N_NEIGH, GCOLS], I8, kind="ExternalInput"
    ).ap()
    out = nc.dram_tensor(
        "out", [128, SPP * OUT_F], F32, kind="ExternalOutput"
    ).ap()

    with tile.TileContext(nc) as tc:
        with (
            tc.tile_pool(name="persist", bufs=1) as persist,
            tc.tile_pool(name="dram", bufs=1, space="DRAM") as dram,
        ):
            ybig = dram.tile([T_ROWS, 1024], F16)
            half_sb = persist.tile([128, 1], F32)
            nc.vector.memset(half_sb[:], 0.5)

            # ---------------- phase 1: pair-interleaved Y table
            with (
                tc.tile_pool(name="p1", bufs=1) as p1,
                tc.tile_pool(name="p1y", bufs=3) as p1y,
                tc.tile_pool(name="p1ps", bufs=2, space="PSUM") as p1ps,
            ):
                x2t_sb = p1.tile([128, T_ROWS], F16)
                nc.sync.dma_start(out=x2t_sb[:], in_=x2t[:])
                rhs2_sb = p1.tile([128, 1024], F16)
                nc.sync.dma_start(out=rhs2_sb[:], in_=rhs2[:])
                bias_sb = p1.tile([128, 1024], F32)
                nc.sync.dma_start(out=bias_sb[:], in_=bias[:])

                for j in range(NBLK):
                    psum = p1ps.tile([128, 1024], F32, space="PSUM", tag="ps")
                    lhsT = x2t_sb[:, j * 128 : (j + 1) * 128]
                    nc.tensor.matmul(
                        out=psum[:, 0:512],
                        lhsT=lhsT,
                        rhs=rhs2_sb[:, 0:512],
                        start=True,
                        stop=True,
                    )
                    nc.tensor.matmul(
                        out=psum[:, 512:1024],
                        lhsT=lhsT,
                        rhs=rhs2_sb[:, 512:1024],
                        start=True,
                        stop=True,
                    )
                    y_sb = p1y.tile([128, 1024], F16, tag="y")
                    nc.vector.tensor_tensor(
                        out=y_sb[:],
                        in0=psum[:],
                        in1=bias_sb[:],
                        op=mybir.AluOpType.add,
                    )
                    nc.sync.dma_start(
                        out=ybig[j * 128 : (j + 1) * 128, :], in_=y_sb[:]
                    )

            # ---------------- phase 2: single-bank pair gather + select
            with (
                tc.tile_pool(name="p2", bufs=2) as p2,
                tc.tile_pool(name="p2s", bufs=2) as p2s,
            ):
                x1 = None
                for j in range(N_CHUNKS):
                    idx_sb = p2.tile([128, N_NEIGH, NIDX // 16], I16, tag="idx")
                    nc.sync.dma_start(
                        out=idx_sb[:],
                        in_=idx[j]
                        .rearrange("(o p) k n -> o p k n", o=1)
                        .to_broadcast([8, 16, N_NEIGH, NIDX // 16]),
                    )
                    m_sb = p2.tile([128, N_NEIGH, GCOLS], I8, tag="m")
                    nc.sync.dma_start(out=m_sb[:], in_=msk[j])

                    g = p2.tile([128, N_NEIGH, GCOLS, 128], F16, tag="g")
                    for k in range(N_NEIGH):
                        nc.gpsimd.dma_gather(
                            out_ap=g[:, k, :, :],
                            in_ap=ybig[:, k * 128 : (k + 1) * 128],
                            idxs_ap=idx_sb[:, k, :],
                            num_idxs=NIDX,
                            num_idxs_reg=NIDX,
                            elem_size=128,
                            elem_step=1024,
                        )
                    # pair-half select (per-chunk sel tile)
                    sel = p2.tile([128, N_NEIGH, GCOLS, 72], F16, tag="sel")
                    sel_out = sel[:, :, :, 0:64].rearrange("p k c f -> p (k c) f")
                    nc.vector.tensor_copy(sel_out, g[:, :, :, 64:128])
                    nc.vector.copy_predicated(
                        sel_out,
                        m_sb[:]
                        .rearrange("p k c -> p (k c)")
                        .rearrange("p (m o) -> p m o", o=1)
                        .to_broadcast([128, N_NEIGH * GCOLS, 64]),
                        g[:, :, :, 0:64],
                    )
                    if j % 3 == 0:
                        x1 = p2s.tile([128, RCOLS, 64], F32, tag="x1")
                    sub = j % 3
                    nc.vector.tensor_reduce(
                        out=x1[:, sub * GCOLS : (sub + 1) * GCOLS, :],
                        in_=sel[:, :, :, 0:64].rearrange("p k c f -> p c f k"),
                        axis=mybir.AxisListType.X,
                        op=mybir.AluOpType.add,
                    )
                    if sub != 2:
                        continue
                    grp = j // 3  # covers sites 2*grp, 2*grp+1 per partition
                    # softplus(x) - ln2 == Ln(0.5*Exp(x) + 0.5)
                    x2 = p2s.tile([128, RCOLS, 64], F32, tag="x2")
                    nc.scalar.activation(
                        out=x2[:], in_=x1[:], func=mybir.ActivationFunctionType.Exp
                    )
                    nc.scalar.activation(
                        out=x2[:],
                        in_=x2[:],
                        func=mybir.ActivationFunctionType.Ln,
                        scale=0.5,
                        bias=half_sb[:],
                    )
                    acc = p2s.tile([128, 2, 64], F32, tag="acc")
                    nc.vector.tensor_reduce(
                        out=acc[:],
                        in_=x2[:].rearrange("p (s q) f -> p s f q", q=N_PERM),
                        axis=mybir.AxisListType.X,
                        op=mybir.AluOpType.add,
                    )
                    nc.sync.dma_start(
                        out=out[:, grp * 128 : grp * 128 + 128],
                        in_=acc[:].rearrange("p s f -> p (s f)"),
                    )

    nc.compile()
    return nc


def _host_prep(X_sites, X_NSs, W, b):
    X_sites = np.asarray(X_sites, dtype=np.float32)
    X_NSs = np.asarray(X_NSs)
    W = np.asarray(W, dtype=np.float32)
    b = np.asarray(b, dtype=np.float32)

    Xp = np.zeros((SITES_PAD, NODE_F), dtype=np.float16)
    Xp[:N_SITES] = X_sites.astype(np.float16)
    x2t = np.ascontiguousarray(
        Xp.reshape(T_ROWS, 2, NODE_F).transpose(1, 2, 0).reshape(128, T_ROWS)
    )

    Wk = W.reshape(OUT_F, N_NEIGH, NODE_F)  # [o, k, f']
    rhs2 = np.zeros((128, 1024), dtype=np.float16)
    for par in range(2):
        for k in range(N_NEIGH):
            c0 = k * 128 + par * 64
            rhs2[par * 64 : par * 64 + 64, c0 : c0 + 64] = Wk[:, k, :].T.astype(
                np.float16
            )
    bias = np.zeros((128, 1024), dtype=np.float32)
    for par in range(2):
        c0 = 7 * 128 + par * 64
        bias[:, c0 : c0 + 64] = b[None, :]

    in_maps = []
    for c in range(N_CORES):
        ns = X_NSs[c * SITES_PER_CORE : (c + 1) * SITES_PER_CORE]
        nsp = np.zeros((PAD_SITES_CORE, N_PERM, N_NEIGH), dtype=np.int64)
        nsp[:SITES_PER_CORE] = ns
        sites = nsp.reshape(128, SPP, N_PERM, N_NEIGH)  # [p, s, q, k]
        t = (sites >> 1).astype(np.int16)
        par = (sites & 1).astype(np.int8)
        # global col ordering per partition: (s, q) -> s*12+q, split into
        # chunks of GCOLS; position i = c8*128 + p
        arr = (
            t.reshape(128, SPP * N_PERM, N_NEIGH)
            .transpose(1, 2, 0)  # [col, k, p]
            .reshape(N_CHUNKS, GCOLS, N_NEIGH, 128)
            .transpose(0, 2, 1, 3)  # [chunk, k, c8, p]
            .reshape(N_CHUNKS, N_NEIGH, NIDX)
        )
        idxv = np.ascontiguousarray(
            arr.reshape(N_CHUNKS, N_NEIGH, NIDX // 16, 16).transpose(0, 3, 1, 2)
        )
        mskv = np.ascontiguousarray(
            (1 - par)
            .reshape(128, SPP * N_PERM, N_NEIGH)
            .transpose(1, 2, 0)  # [col, k, p]
            .reshape(N_CHUNKS, GCOLS, N_NEIGH, 128)
            .transpose(0, 3, 2, 1)  # [chunk, p, k, c8]
            .astype(np.int8)
        )
        in_maps.append(
            {"x2t": x2t, "rhs2": rhs2, "bias": bias, "idx": idxv, "msk": mskv}
        )
    return in_maps


_NC_CACHE = {}


def _get_nc():
    if "nc" not in _NC_CACHE:
        _NC_CACHE["nc"] = build_nc()
    return _NC_CACHE["nc"]


def _stitch(results):
    full = np.empty((N_SITES, OUT_F), dtype=np.float32)
    for c, r in enumerate(results):
        o = r["out"].reshape(PAD_SITES_CORE, OUT_F)[:SITES_PER_CORE]
        full[c * SITES_PER_CORE : (c + 1) * SITES_PER_CORE] = o
    return full


def kernel(X_sites, X_NSs, W, b, _trace=False):
    nc = _get_nc()
    in_maps = _host_prep(X_sites, X_NSs, W, b)
    res = run_bass_kernel_spmd(
        nc, in_maps, core_ids=list(range(N_CORES)), trace=_trace
    )
    full = _stitch(res.results)
    if _trace:
        return full, res
    return full


# revision 6
# speedup vs baseline: 5.0107x; 1.8224x over previous
"""v2c: same single-bank pair-table design, but num_idxs=1024 per gather call
(the baseline-proven call size). Gather chunks cover 8 columns; select lands
in a 24-col (2-site) sel buffer; slot-reduce/softplus/perm-reduce fire every
3rd chunk. SPP=50 (pad 6400 sites/core), 75 chunks, 600 gather calls.
"""

import numpy as np

import concourse.bass as bass
import concourse.bacc as bacc
import concourse.mybir as mybir
import concourse.tile as tile
from concourse.bass_utils import run_bass_kernel_spmd

N_SITES = 50000
NODE_F = 64
N_PERM = 12
N_NEIGH = 8
OUT_F = 64

N_CORES = 8
SITES_PER_CORE = N_SITES // N_CORES            # 6250
SPP = 50                                       # sites per partition (pad 6400)
PAD_SITES_CORE = 128 * SPP                     # 6400

SITES_PAD = 50176                              # 2 * 25088 (table pad)
T_ROWS = SITES_PAD // 2                        # 25088 pair rows
NBLK = T_ROWS // 128                           # 196 phase-1 blocks

GCOLS = 8                                      # gather cols per partition/call
NIDX = 128 * GCOLS                             # 1024
RCOLS = 24                                     # reduce group = 2 sites
N_CHUNKS = SPP * N_PERM // GCOLS               # 75

F32 = mybir.dt.float32
F16 = mybir.dt.float16
I16 = mybir.dt.int16
I8 = mybir.dt.int8


def build_nc():
    nc = bacc.Bacc("TRN2", target_bir_lowering=False, debug=False)

    x2t = nc.dram_tensor("x2t", [16, T_ROWS], F16, kind="ExternalInput").ap()
    rhs2 = nc.dram_tensor("rhs2", [128, 1024], F16, kind="ExternalInput").ap()
    bias = nc.dram_tensor("bias", [128, 1024], F32, kind="ExternalInput").ap()
    idx = nc.dram_tensor(
        "idx", [N_CHUNKS, 16, N_NEIGH, NIDX // 16], I16, kind="ExternalInput"
    ).ap()
    msk = nc.dram_tensor(
        "msk", [N_CHUNKS, 128, N_NEIGH, GCOLS], I8, kind="ExternalInput"
    ).ap()
    out = nc.dram_tensor(
        "out", [128, SPP * OUT_F], F32, kind="ExternalOutput"
    ).ap()

    with tile.TileContext(nc) as tc:
        with (
            tc.tile_pool(name="persist", bufs=1) as persist,
            tc.tile_pool(name="dram", bufs=1, space="DRAM") as dram,
        ):
            ybig = dram.tile([T_ROWS, 1024], F16)
            half_sb = persist.tile([128, 1], F32)
            nc.vector.memset(half_sb[:], 0.5)

            # all-gather the pair-interleaved X.T from per-core shards
            x2t_in = dram.tile([16, T_ROWS], F16)
            x2t_full = dram.tile([128, T_ROWS], F16)
            with tc.tile_pool(name="p0", bufs=1) as p0:
                sh_sb = p0.tile([16, T_ROWS], F16)
                nc.sync.dma_start(out=sh_sb[:], in_=x2t[:])
                nc.sync.dma_start(out=x2t_in[:], in_=sh_sb[:])
                nc.gpsimd.collective_compute(
                    "AllGather",
                    mybir.AluOpType.bypass,
                    replica_groups=[list(range(8))],
                    ins=[x2t_in.opt()],
                    outs=[x2t_full.opt()],
                )

            # ---------------- phase 1: pair-interleaved Y table
            with (
                tc.tile_pool(name="p1", bufs=1) as p1,
                tc.tile_pool(name="p1y", bufs=3) as p1y,
                tc.tile_pool(name="p1ps", bufs=2, space="PSUM") as p1ps,
            ):
                x2t_sb = p1.tile([128, T_ROWS], F16)
                nc.sync.dma_start(out=x2t_sb[:], in_=x2t_full[:])
                rhs2_sb = p1.tile([128, 1024], F16)
                nc.sync.dma_start(out=rhs2_sb[:], in_=rhs2[:])
                bias_sb = p1.tile([128, 1024], F32)
                nc.sync.dma_start(out=bias_sb[:], in_=bias[:])

                for j in range(NBLK):
                    psum = p1ps.tile([128, 1024], F32, space="PSUM", tag="ps")
                    lhsT = x2t_sb[:, j * 128 : (j + 1) * 128]
                    nc.tensor.matmul(
                        out=psum[:, 0:512],
                        lhsT=lhsT,
                        rhs=rhs2_sb[:, 0:512],
                        start=True,
                        stop=True,
                    )
                    nc.tensor.matmul(
                        out=psum[:, 512:1024],
                        lhsT=lhsT,
                        rhs=rhs2_sb[:, 512:1024],
                        start=True,
                        stop=True,
                    )
                    y_sb = p1y.tile([128, 1024], F16, tag="y")
                    nc.vector.tensor_tensor(
                        out=y_sb[:],
                        in0=psum[:],
                        in1=bias_sb[:],
                        op=mybir.AluOpType.add,
                    )
                    nc.sync.dma_start(
                        out=ybig[j * 128 : (j + 1) * 128, :], in_=y_sb[:]
                    )

            # ---------------- phase 2: single-bank pair gather + select
            with (
                tc.tile_pool(name="p2", bufs=2) as p2,
                tc.tile_pool(name="p2s", bufs=2) as p2s,
            ):
                x1 = None
                for j in range(N_CHUNKS):
                    idx_sb = p2.tile([128, N_NEIGH, NIDX // 16], I16, tag="idx")
                    nc.sync.dma_start(
                        out=idx_sb[:],
                        in_=idx[j]
                        .rearrange("(o p) k n -> o p k n", o=1)
                        .to_broadcast([8, 16, N_NEIGH, NIDX // 16]),
                    )
                    m_sb = p2.tile([128, N_NEIGH, GCOLS], I8, tag="m")
                    nc.sync.dma_start(out=m_sb[:], in_=msk[j])

                    g = p2.tile([128, N_NEIGH, GCOLS, 128], F16, tag="g")
                    for k in range(N_NEIGH):
                        nc.gpsimd.dma_gather(
                            out_ap=g[:, k, :, :],
                            in_ap=ybig[:, k * 128 : (k + 1) * 128],
                            idxs_ap=idx_sb[:, k, :],
                            num_idxs=NIDX,
                            num_idxs_reg=NIDX,
                            elem_size=128,
                            elem_step=1024,
                        )
                    # pair-half select (per-chunk sel tile)
                    sel = p2.tile([128, N_NEIGH, GCOLS, 72], F16, tag="sel")
                    sel_out = sel[:, :, :, 0:64].rearrange("p k c f -> p (k c) f")
                    nc.vector.tensor_copy(sel_out, g[:, :, :, 64:128])
                    nc.vector.copy_predicated(
                        sel_out,
                        m_sb[:]
                        .rearrange("p k c -> p (k c)")
                        .rearrange("p (m o) -> p m o", o=1)
                        .to_broadcast([128, N_NEIGH * GCOLS, 64]),
                        g[:, :, :, 0:64],
                    )
                    if j % 3 == 0:
                        x1 = p2s.tile([128, RCOLS, 64], F32, tag="x1")
                    sub = j % 3
                    nc.vector.tensor_reduce(
                        out=x1[:, sub * GCOLS : (sub + 1) * GCOLS, :],
                        in_=sel[:, :, :, 0:64].rearrange("p k c f -> p c f k"),
                        axis=mybir.AxisListType.X,
                        op=mybir.AluOpType.add,
                    )
                    if sub != 2:
                        continue
                    grp = j // 3  # covers sites 2*grp, 2*grp+1 per partition
                    # softplus(x) - ln2 == Ln(0.5*Exp(x) + 0.5)
                    x2 = p2s.tile([128, RCOLS, 64], F32, tag="x2")
                    nc.scalar.activation(
                        out=x2[:], in_=x1[:], func=mybir.ActivationFunctionType.Exp
                    )
                    nc.scalar.activation(
                        out=x2[:],
                        in_=x2[:],
                        func=mybir.ActivationFunctionType.Ln,
                        scale=0.5,
                        bias=half_sb[:],
                    )
                    acc = p2s.tile([128, 2, 64], F32, tag="acc")
                    nc.vector.tensor_reduce(
                        out=acc[:],
                        in_=x2[:].rearrange("p (s q) f -> p s f q", q=N_PERM),
                        axis=mybir.AxisListType.X,
                        op=mybir.AluOpType.add,
                    )
                    nc.sync.dma_start(
                        out=out[:, grp * 128 : grp * 128 + 128],
                        in_=acc[:].rearrange("p s f -> p (s f)"),
                    )

    nc.compile()
    return nc


def _host_prep(X_sites, X_NSs, W, b):
    X_sites = np.asarray(X_sites, dtype=np.float32)
    X_NSs = np.asarray(X_NSs)
    W = np.asarray(W, dtype=np.float32)
    b = np.asarray(b, dtype=np.float32)

    Xp = np.zeros((SITES_PAD, NODE_F), dtype=np.float16)
    Xp[:N_SITES] = X_sites.astype(np.float16)
    x2t = np.ascontiguousarray(
        Xp.reshape(T_ROWS, 2, NODE_F).transpose(1, 2, 0).reshape(128, T_ROWS)
    )

    Wk = W.reshape(OUT_F, N_NEIGH, NODE_F)  # [o, k, f']
    rhs2 = np.zeros((128, 1024), dtype=np.float16)
    for par in range(2):
        for k in range(N_NEIGH):
            c0 = k * 128 + par * 64
            rhs2[par * 64 : par * 64 + 64, c0 : c0 + 64] = Wk[:, k, :].T.astype(
                np.float16
            )
    bias = np.zeros((128, 1024), dtype=np.float32)
    for par in range(2):
        c0 = 7 * 128 + par * 64
        bias[:, c0 : c0 + 64] = b[None, :]

    in_maps = []
    for c in range(N_CORES):
        ns = X_NSs[c * SITES_PER_CORE : (c + 1) * SITES_PER_CORE]
        nsp = np.zeros((PAD_SITES_CORE, N_PERM, N_NEIGH), dtype=np.int64)
        nsp[:SITES_PER_CORE] = ns
        sites = nsp.reshape(128, SPP, N_PERM, N_NEIGH)  # [p, s, q, k]
        t = (sites >> 1).astype(np.int16)
        par = (sites & 1).astype(np.int8)
        # global col ordering per partition: (s, q) -> s*12+q, split into
        # chunks of GCOLS; position i = c8*128 + p
        arr = (
            t.reshape(128, SPP * N_PERM, N_NEIGH)
            .transpose(1, 2, 0)  # [col, k, p]
            .reshape(N_CHUNKS, GCOLS, N_NEIGH, 128)
            .transpose(0, 2, 1, 3)  # [chunk, k, c8, p]
            .reshape(N_CHUNKS, N_NEIGH, NIDX)
        )
        idxv = np.ascontiguousarray(
            arr.reshape(N_CHUNKS, N_NEIGH, NIDX // 16, 16).transpose(0, 3, 1, 2)
        )
        mskv = np.ascontiguousarray(
            (1 - par)
            .reshape(128, SPP * N_PERM, N_NEIGH)
            .transpose(1, 2, 0)  # [col, k, p]
            .reshape(N_CHUNKS, GCOLS, N_NEIGH, 128)
            .transpose(0, 3, 2, 1)  # [chunk, p, k, c8]
            .astype(np.int8)
        )
        in_maps.append(
            {
                "x2t": np.ascontiguousarray(x2t[16 * c : 16 * (c + 1)]),
                "rhs2": rhs2,
                "bias": bias,
                "idx": idxv,
                "msk": mskv,
            }
        )
    return in_maps


_NC_CACHE = {}


def _get_nc():
    if "nc" not in _NC_CACHE:
        _NC_CACHE["nc"] = build_nc()
    return _NC_CACHE["nc"]


def _stitch(results):
    full = np.empty((N_SITES, OUT_F), dtype=np.float32)
    for c, r in enumerate(results):
        o = r["out"].reshape(PAD_SITES_CORE, OUT_F)[:SITES_PER_CORE]
        full[c * SITES_PER_CORE : (c + 1) * SITES_PER_CORE] = o
    return full


def kernel(X_sites, X_NSs, W, b, _trace=False):
    nc = _get_nc()
    in_maps = _host_prep(X_sites, X_NSs, W, b)
    res = run_bass_kernel_spmd(
        nc, in_maps, core_ids=list(range(N_CORES)), trace=_trace
    )
    full = _stitch(res.results)
    if _trace:
        return full, res
    return full


# revision 7
# speedup vs baseline: 7.6441x; 1.5256x over previous
"""v2c: same single-bank pair-table design, but num_idxs=1024 per gather call
(the baseline-proven call size). Gather chunks cover 8 columns; select lands
in a 24-col (2-site) sel buffer; slot-reduce/softplus/perm-reduce fire every
3rd chunk. SPP=50 (pad 6400 sites/core), 75 chunks, 600 gather calls.
"""

import numpy as np

import concourse.bass as bass
import concourse.bacc as bacc
import concourse.mybir as mybir
import concourse.tile as tile
from concourse.bass_utils import run_bass_kernel_spmd

N_SITES = 50000
NODE_F = 64
N_PERM = 12
N_NEIGH = 8
OUT_F = 64

N_CORES = 8
SITES_PER_CORE = N_SITES // N_CORES            # 6250
SPP = 50                                       # sites per partition (pad 6400)
PAD_SITES_CORE = 128 * SPP                     # 6400

SITES_PAD = 50176                              # 2 * 25088 (table pad)
T_ROWS = SITES_PAD // 2                        # 25088 pair rows
NBLK = T_ROWS // 128                           # 196 phase-1 blocks

GCOLS = 8                                      # gather cols per partition/call
NIDX = 128 * GCOLS                             # 1024
RCOLS = 24                                     # reduce group = 2 sites
N_CHUNKS = SPP * N_PERM // GCOLS               # 75

F32 = mybir.dt.float32
F16 = mybir.dt.float16
I16 = mybir.dt.int16
I8 = mybir.dt.int8


def build_nc():
    nc = bacc.Bacc("TRN2", target_bir_lowering=False, debug=False)

    x2t = nc.dram_tensor("x2t", [16, T_ROWS], F16, kind="ExternalInput").ap()
    rhs2 = nc.dram_tensor("rhs2", [128, 1024], F16, kind="ExternalInput").ap()
    bias = nc.dram_tensor("bias", [1, 1024], F32, kind="ExternalInput").ap()
    idx = nc.dram_tensor(
        "idx", [N_CHUNKS, 16, N_NEIGH, NIDX // 16], I16, kind="ExternalInput"
    ).ap()
    msk = nc.dram_tensor(
        "msk", [N_CHUNKS, 128, N_NEIGH, GCOLS], I8, kind="ExternalInput"
    ).ap()
    out = nc.dram_tensor(
        "out", [128, SPP * OUT_F], F16, kind="ExternalOutput"
    ).ap()

    with tile.TileContext(nc) as tc:
        with (
            tc.tile_pool(name="persist", bufs=1) as persist,
            tc.tile_pool(name="dram", bufs=1, space="DRAM") as dram,
        ):
            ybig = dram.tile([T_ROWS, 1024], F16)
            half_sb = persist.tile([128, 1], F32)
            nc.vector.memset(half_sb[:], 0.5)

            # all-gather the pair-interleaved X.T from per-core shards
            x2t_in = dram.tile([16, T_ROWS], F16)
            x2t_full = dram.tile([128, T_ROWS], F16)
            with tc.tile_pool(name="p0", bufs=1) as p0:
                sh_sb = p0.tile([16, T_ROWS], F16)
                nc.sync.dma_start(out=sh_sb[:], in_=x2t[:])
                nc.sync.dma_start(out=x2t_in[:], in_=sh_sb[:])
                nc.gpsimd.collective_compute(
                    "AllGather",
                    mybir.AluOpType.bypass,
                    replica_groups=[list(range(8))],
                    ins=[x2t_in.opt()],
                    outs=[x2t_full.opt()],
                )

            # ---------------- phase 1: pair-interleaved Y table
            with (
                tc.tile_pool(name="p1", bufs=1) as p1,
                tc.tile_pool(name="p1y", bufs=3) as p1y,
                tc.tile_pool(name="p1ps", bufs=2, space="PSUM") as p1ps,
            ):
                x2t_sb = p1.tile([128, T_ROWS], F16)
                nc.sync.dma_start(out=x2t_sb[:], in_=x2t_full[:])
                rhs2_sb = p1.tile([128, 1024], F16)
                nc.sync.dma_start(out=rhs2_sb[:], in_=rhs2[:])
                bias_row = p1.tile([1, 1024], F32)
                nc.sync.dma_start(out=bias_row[:], in_=bias[:])
                bias_sb = p1.tile([128, 1024], F32)
                nc.gpsimd.partition_broadcast(
                    out_ap=bias_sb[:], in_ap=bias_row[:]
                )

                for j in range(NBLK):
                    psum = p1ps.tile([128, 1024], F32, space="PSUM", tag="ps")
                    lhsT = x2t_sb[:, j * 128 : (j + 1) * 128]
                    nc.tensor.matmul(
                        out=psum[:, 0:512],
                        lhsT=lhsT,
                        rhs=rhs2_sb[:, 0:512],
                        start=True,
                        stop=True,
                    )
                    nc.tensor.matmul(
                        out=psum[:, 512:1024],
                        lhsT=lhsT,
                        rhs=rhs2_sb[:, 512:1024],
                        start=True,
                        stop=True,
                    )
                    y_sb = p1y.tile([128, 1024], F16, tag="y")
                    nc.vector.tensor_tensor(
                        out=y_sb[:],
                        in0=psum[:],
                        in1=bias_sb[:],
                        op=mybir.AluOpType.add,
                    )
                    nc.sync.dma_start(
                        out=ybig[j * 128 : (j + 1) * 128, :], in_=y_sb[:]
                    )

            # ---------------- phase 2: single-bank pair gather + select
            with (
                tc.tile_pool(name="p2", bufs=2) as p2,
                tc.tile_pool(name="p2s", bufs=2) as p2s,
            ):
                x1 = None
                for j in range(N_CHUNKS):
                    idx_sb = p2.tile([128, N_NEIGH, NIDX // 16], I16, tag="idx")
                    nc.sync.dma_start(
                        out=idx_sb[:],
                        in_=idx[j]
                        .rearrange("(o p) k n -> o p k n", o=1)
                        .to_broadcast([8, 16, N_NEIGH, NIDX // 16]),
                    )
                    m_sb = p2.tile([128, N_NEIGH, GCOLS], I8, tag="m")
                    nc.sync.dma_start(out=m_sb[:], in_=msk[j])

                    g = p2.tile([128, N_NEIGH, GCOLS, 128], F16, tag="g")
                    for k in range(N_NEIGH):
                        nc.gpsimd.dma_gather(
                            out_ap=g[:, k, :, :],
                            in_ap=ybig[:, k * 128 : (k + 1) * 128],
                            idxs_ap=idx_sb[:, k, :],
                            num_idxs=NIDX,
                            num_idxs_reg=NIDX,
                            elem_size=128,
                            elem_step=1024,
                        )
                    # pair-half select (per-chunk sel tile)
                    sel = p2.tile([128, N_NEIGH, GCOLS, 72], F16, tag="sel")
                    sel_out = sel[:, :, :, 0:64].rearrange("p k c f -> p (k c) f")
                    nc.vector.tensor_copy(sel_out, g[:, :, :, 64:128])
                    nc.vector.copy_predicated(
                        sel_out,
                        m_sb[:]
                        .rearrange("p k c -> p (k c)")
                        .rearrange("p (m o) -> p m o", o=1)
                        .to_broadcast([128, N_NEIGH * GCOLS, 64]),
                        g[:, :, :, 0:64],
                    )
                    if j % 3 == 0:
                        x1 = p2s.tile([128, RCOLS, 64], F32, tag="x1")
                    sub = j % 3
                    nc.vector.tensor_reduce(
                        out=x1[:, sub * GCOLS : (sub + 1) * GCOLS, :],
                        in_=sel[:, :, :, 0:64].rearrange("p k c f -> p c f k"),
                        axis=mybir.AxisListType.X,
                        op=mybir.AluOpType.add,
                    )
                    if sub != 2:
                        continue
                    grp = j // 3  # covers sites 2*grp, 2*grp+1 per partition
                    # softplus(x) - ln2 == Ln(0.5*Exp(x) + 0.5)
                    x2 = p2s.tile([128, RCOLS, 64], F32, tag="x2")
                    nc.scalar.activation(
                        out=x2[:], in_=x1[:], func=mybir.ActivationFunctionType.Exp
                    )
                    nc.scalar.activation(
                        out=x2[:],
                        in_=x2[:],
                        func=mybir.ActivationFunctionType.Ln,
                        scale=0.5,
                        bias=half_sb[:],
                    )
                    acc = p2s.tile([128, 2, 64], F16, tag="acc")
                    with nc.allow_low_precision(reason="12-term softplus sum"):
                        nc.vector.tensor_reduce(
                            out=acc[:],
                            in_=x2[:].rearrange("p (s q) f -> p s f q", q=N_PERM),
                            axis=mybir.AxisListType.X,
                            op=mybir.AluOpType.add,
                        )
                    nc.sync.dma_start(
                        out=out[:, grp * 128 : grp * 128 + 128],
                        in_=acc[:].rearrange("p s f -> p (s f)"),
                    )

    nc.compile()
    return nc


def _host_prep(X_sites, X_NSs, W, b):
    X_sites = np.asarray(X_sites, dtype=np.float32)
    X_NSs = np.asarray(X_NSs)
    W = np.asarray(W, dtype=np.float32)
    b = np.asarray(b, dtype=np.float32)

    Xp = np.zeros((SITES_PAD, NODE_F), dtype=np.float16)
    Xp[:N_SITES] = X_sites.astype(np.float16)
    x2t = np.ascontiguousarray(
        Xp.reshape(T_ROWS, 2, NODE_F).transpose(1, 2, 0).reshape(128, T_ROWS)
    )

    Wk = W.reshape(OUT_F, N_NEIGH, NODE_F)  # [o, k, f']
    rhs2 = np.zeros((128, 1024), dtype=np.float16)
    for par in range(2):
        for k in range(N_NEIGH):
            c0 = k * 128 + par * 64
            rhs2[par * 64 : par * 64 + 64, c0 : c0 + 64] = Wk[:, k, :].T.astype(
                np.float16
            )
    bias = np.zeros((1, 1024), dtype=np.float32)
    for par in range(2):
        c0 = 7 * 128 + par * 64
        bias[0, c0 : c0 + 64] = b

    in_maps = []
    for c in range(N_CORES):
        ns = X_NSs[c * SITES_PER_CORE : (c + 1) * SITES_PER_CORE]
        nsp = np.zeros((PAD_SITES_CORE, N_PERM, N_NEIGH), dtype=np.int64)
        nsp[:SITES_PER_CORE] = ns
        sites = nsp.reshape(128, SPP, N_PERM, N_NEIGH)  # [p, s, q, k]
        t = (sites >> 1).astype(np.int16)
        par = (sites & 1).astype(np.int8)
        # global col ordering per partition: (s, q) -> s*12+q, split into
        # chunks of GCOLS; position i = c8*128 + p
        arr = (
            t.reshape(128, SPP * N_PERM, N_NEIGH)
            .transpose(1, 2, 0)  # [col, k, p]
            .reshape(N_CHUNKS, GCOLS, N_NEIGH, 128)
            .transpose(0, 2, 1, 3)  # [chunk, k, c8, p]
            .reshape(N_CHUNKS, N_NEIGH, NIDX)
        )
        idxv = np.ascontiguousarray(
            arr.reshape(N_CHUNKS, N_NEIGH, NIDX // 16, 16).transpose(0, 3, 1, 2)
        )
        mskv = np.ascontiguousarray(
            (1 - par)
            .reshape(128, SPP * N_PERM, N_NEIGH)
            .transpose(1, 2, 0)  # [col, k, p]
            .reshape(N_CHUNKS, GCOLS, N_NEIGH, 128)
            .transpose(0, 3, 2, 1)  # [chunk, p, k, c8]
            .astype(np.int8)
        )
        in_maps.append(
            {
                "x2t": np.ascontiguousarray(x2t[16 * c : 16 * (c + 1)]),
                "rhs2": rhs2,
                "bias": bias,
                "idx": idxv,
                "msk": mskv,
            }
        )
    return in_maps


_NC_CACHE = {}


def _get_nc():
    if "nc" not in _NC_CACHE:
        _NC_CACHE["nc"] = build_nc()
    return _NC_CACHE["nc"]


def _stitch(results):
    full = np.empty((N_SITES, OUT_F), dtype=np.float32)
    for c, r in enumerate(results):
        o = r["out"].astype(np.float32).reshape(PAD_SITES_CORE, OUT_F)[:SITES_PER_CORE]
        full[c * SITES_PER_CORE : (c + 1) * SITES_PER_CORE] = o
    return full


def kernel(X_sites, X_NSs, W, b, _trace=False):
    nc = _get_nc()
    in_maps = _host_prep(X_sites, X_NSs, W, b)
    res = run_bass_kernel_spmd(
        nc, in_maps, core_ids=list(range(N_CORES)), trace=_trace
    )
    full = _stitch(res.results)
    if _trace:
        return full, res
    return full


# revision 8
# speedup vs baseline: 17.1595x; 2.2448x over previous
"""Trainium2 Bass kernel for nn_LCNNConvolution (GNN message passing), v4.

Math:  out[n] = sum_p softplus( gather(X, NS[n,p,:]).flat @ W.T + b ) - 12*ln2
Key transform: W is block-structured over the 8 neighbor slots, so
    x1[n,p,:] = sum_k Y_k[NS[n,p,k]]  with  Y_k = X @ W_k.T  (b baked in slot 7).

Design (vs the 2-bank v1): the Y table stores SITE PAIRS per row - row t
holds, for each slot k, [Y_k[2t], Y_k[2t+1]] contiguously (128 f16 = 256B per
slot). 25088 rows fit int16 gather indices in ONE bank, so the hot loop
issues exactly ONE 256B dma_gather descriptor per (n,p,k) lookup (v1 needed
two banks = 2x descriptors, each half-wasted on a zero-row dummy). num_idxs
is capped at the HW-safe 1024/call. The pair half is chosen on DVE via
copy_predicated with a per-lookup int8 parity mask (free-dim stride-0
broadcast AP; output AP rank-matched via a padded-(k c)-merge). The
pair-interleaved table is produced directly by the phase-1 matmul: lhsT =
pair-interleaved X.T (contraction dim = 2 sites x 64 feats), rhs =
block-diagonal-by-parity weights [128, 1024], so no on-chip transpose or
strided DRAM writes are needed.

Upload minimization (wall time through the axon tunnel is transfer-bound):
X.T is uploaded as a per-core 1/8 shard and AllGathered on-device across the
8 cores; bias is a single row partition_broadcast on-device; the parity mask
is int8; the output is f16. ~2.9 MB/core of inputs per run vs ~33 MB for v1.

Sharding: data-parallel over sites; each of the 8 cores handles 6250 sites
(50 sites per partition, perm-major column order) and computes its own full
pair-interleaved Y table from the AllGathered X (replicated W).
"""

import numpy as np

import concourse.bass as bass
import concourse.bacc as bacc
import concourse.mybir as mybir
import concourse.tile as tile
from concourse.bass_utils import run_bass_kernel_spmd

N_SITES = 50000
NODE_F = 64
N_PERM = 12
N_NEIGH = 8
OUT_F = 64

N_CORES = 8
SITES_PER_CORE = N_SITES // N_CORES            # 6250
SPP = 50                                       # sites per partition (pad 6400)
PAD_SITES_CORE = 128 * SPP                     # 6400

SITES_PAD = 50176                              # 2 * 25088 (table pad)
T_ROWS = SITES_PAD // 2                        # 25088 pair rows
NBLK = T_ROWS // 128                           # 196 phase-1 blocks

GCOLS = 8                                      # gather cols per partition/call
NIDX = 128 * GCOLS                             # 1024
RCOLS = 24                                     # reduce group = 2 sites
N_CHUNKS = SPP * N_PERM // GCOLS               # 75

F32 = mybir.dt.float32
F16 = mybir.dt.float16
I16 = mybir.dt.int16
I8 = mybir.dt.int8


def build_nc():
    nc = bacc.Bacc("TRN2", target_bir_lowering=False, debug=False)

    x2t = nc.dram_tensor("x2t", [16, T_ROWS], F16, kind="ExternalInput").ap()
    rhs2 = nc.dram_tensor("rhs2", [128, 1024], F16, kind="ExternalInput").ap()
    bias = nc.dram_tensor("bias", [1, 1024], F32, kind="ExternalInput").ap()
    idx = nc.dram_tensor(
        "idx", [N_CHUNKS, 16, N_NEIGH, NIDX // 16], I16, kind="ExternalInput"
    ).ap()
    msk = nc.dram_tensor(
        "msk", [N_CHUNKS, 128, N_NEIGH, GCOLS], I8, kind="ExternalInput"
    ).ap()
    out = nc.dram_tensor(
        "out", [128, SPP * OUT_F], F16, kind="ExternalOutput"
    ).ap()

    with tile.TileContext(nc) as tc:
        with (
            tc.tile_pool(name="persist", bufs=1) as persist,
            tc.tile_pool(name="dram", bufs=1, space="DRAM") as dram,
        ):
            ybig = dram.tile([T_ROWS, 1024], F16)
            half_sb = persist.tile([128, 1], F32)
            nc.vector.memset(half_sb[:], 0.5)

            # all-gather the pair-interleaved X.T from per-core shards
            x2t_in = dram.tile([16, T_ROWS], F16)
            x2t_full = dram.tile([128, T_ROWS], F16)
            with tc.tile_pool(name="p0", bufs=1) as p0:
                sh_sb = p0.tile([16, T_ROWS], F16)
                nc.sync.dma_start(out=sh_sb[:], in_=x2t[:])
                nc.sync.dma_start(out=x2t_in[:], in_=sh_sb[:])
                nc.gpsimd.collective_compute(
                    "AllGather",
                    mybir.AluOpType.bypass,
                    replica_groups=[list(range(8))],
                    ins=[x2t_in.opt()],
                    outs=[x2t_full.opt()],
                )

            # ---------------- phase 1: pair-interleaved Y table
            with (
                tc.tile_pool(name="p1", bufs=1) as p1,
                tc.tile_pool(name="p1y", bufs=3) as p1y,
                tc.tile_pool(name="p1ps", bufs=2, space="PSUM") as p1ps,
            ):
                x2t_sb = p1.tile([128, T_ROWS], F16)
                nc.sync.dma_start(out=x2t_sb[:], in_=x2t_full[:])
                rhs2_sb = p1.tile([128, 1024], F16)
                nc.sync.dma_start(out=rhs2_sb[:], in_=rhs2[:])
                bias_row = p1.tile([1, 1024], F32)
                nc.sync.dma_start(out=bias_row[:], in_=bias[:])
                bias_sb = p1.tile([128, 1024], F32)
                nc.gpsimd.partition_broadcast(
                    out_ap=bias_sb[:], in_ap=bias_row[:]
                )

                for j in range(NBLK):
                    psum = p1ps.tile([128, 1024], F32, space="PSUM", tag="ps")
                    lhsT = x2t_sb[:, j * 128 : (j + 1) * 128]
                    nc.tensor.matmul(
                        out=psum[:, 0:512],
                        lhsT=lhsT,
                        rhs=rhs2_sb[:, 0:512],
                        start=True,
                        stop=True,
                    )
                    nc.tensor.matmul(
                        out=psum[:, 512:1024],
                        lhsT=lhsT,
                        rhs=rhs2_sb[:, 512:1024],
                        start=True,
                        stop=True,
                    )
                    y_sb = p1y.tile([128, 1024], F16, tag="y")
                    nc.vector.tensor_tensor(
                        out=y_sb[:],
                        in0=psum[:],
                        in1=bias_sb[:],
                        op=mybir.AluOpType.add,
                    )
                    nc.sync.dma_start(
                        out=ybig[j * 128 : (j + 1) * 128, :], in_=y_sb[:]
                    )

            # ---------------- phase 2: single-bank pair gather + select
            with (
                tc.tile_pool(name="p2", bufs=2) as p2,
                tc.tile_pool(name="p2s", bufs=2) as p2s,
            ):
                x1 = None
                for j in range(N_CHUNKS):
                    idx_sb = p2.tile([128, N_NEIGH, NIDX // 16], I16, tag="idx")
                    nc.sync.dma_start(
                        out=idx_sb[:],
                        in_=idx[j]
                        .rearrange("(o p) k n -> o p k n", o=1)
                        .to_broadcast([8, 16, N_NEIGH, NIDX // 16]),
                    )
                    m_sb = p2.tile([128, N_NEIGH, GCOLS], I8, tag="m")
                    nc.sync.dma_start(out=m_sb[:], in_=msk[j])

                    g = p2.tile([128, N_NEIGH, GCOLS, 128], F16, tag="g")
                    for k in range(N_NEIGH):
                        nc.gpsimd.dma_gather(
                            out_ap=g[:, k, :, :],
                            in_ap=ybig[:, k * 128 : (k + 1) * 128],
                            idxs_ap=idx_sb[:, k, :],
                            num_idxs=NIDX,
                            num_idxs_reg=NIDX,
                            elem_size=128,
                            elem_step=1024,
                        )
                    # pair-half select (per-chunk sel tile)
                    sel = p2.tile([128, N_NEIGH, GCOLS, 72], F16, tag="sel")
                    sel_out = sel[:, :, :, 0:64].rearrange("p k c f -> p (k c) f")
                    nc.vector.tensor_copy(sel_out, g[:, :, :, 64:128])
                    nc.vector.copy_predicated(
                        sel_out,
                        m_sb[:]
                        .rearrange("p k c -> p (k c)")
                        .rearrange("p (m o) -> p m o", o=1)
                        .to_broadcast([128, N_NEIGH * GCOLS, 64]),
                        g[:, :, :, 0:64],
                    )
                    if j % 3 == 0:
                        x1 = p2s.tile([128, RCOLS, 64], F32, tag="x1")
                    sub = j % 3
                    nc.vector.tensor_reduce(
                        out=x1[:, sub * GCOLS : (sub + 1) * GCOLS, :],
                        in_=sel[:, :, :, 0:64].rearrange("p k c f -> p c f k"),
                        axis=mybir.AxisListType.X,
                        op=mybir.AluOpType.add,
                    )
                    if sub != 2:
                        continue
                    grp = j // 3  # covers sites 2*grp, 2*grp+1 per partition
                    # softplus(x) - ln2 == Ln(0.5*Exp(x) + 0.5)
                    x2 = p2s.tile([128, RCOLS, 64], F32, tag="x2")
                    nc.scalar.activation(
                        out=x2[:], in_=x1[:], func=mybir.ActivationFunctionType.Exp
                    )
                    nc.scalar.activation(
                        out=x2[:],
                        in_=x2[:],
                        func=mybir.ActivationFunctionType.Ln,
                        scale=0.5,
                        bias=half_sb[:],
                    )
                    acc = p2s.tile([128, 2, 64], F16, tag="acc")
                    with nc.allow_low_precision(reason="12-term softplus sum"):
                        nc.vector.tensor_reduce(
                            out=acc[:],
                            in_=x2[:].rearrange("p (s q) f -> p s f q", q=N_PERM),
                            axis=mybir.AxisListType.X,
                            op=mybir.AluOpType.add,
                        )
                    nc.sync.dma_start(
                        out=out[:, grp * 128 : grp * 128 + 128],
                        in_=acc[:].rearrange("p s f -> p (s f)"),
                    )

    nc.compile()
    return nc


def _host_prep(X_sites, X_NSs, W, b):
    X_sites = np.asarray(X_sites, dtype=np.float32)
    X_NSs = np.asarray(X_NSs)
    W = np.asarray(W, dtype=np.float32)
    b = np.asarray(b, dtype=np.float32)

    Xp = np.zeros((SITES_PAD, NODE_F), dtype=np.float16)
    Xp[:N_SITES] = X_sites.astype(np.float16)
    x2t = np.ascontiguousarray(
        Xp.reshape(T_ROWS, 2, NODE_F).transpose(1, 2, 0).reshape(128, T_ROWS)
    )

    Wk = W.reshape(OUT_F, N_NEIGH, NODE_F)  # [o, k, f']
    rhs2 = np.zeros((128, 1024), dtype=np.float16)
    for par in range(2):
        for k in range(N_NEIGH):
            c0 = k * 128 + par * 64
            rhs2[par * 64 : par * 64 + 64, c0 : c0 + 64] = Wk[:, k, :].T.astype(
                np.float16
            )
    bias = np.zeros((1, 1024), dtype=np.float32)
    for par in range(2):
        c0 = 7 * 128 + par * 64
        bias[0, c0 : c0 + 64] = b

    in_maps = []
    for c in range(N_CORES):
        ns = X_NSs[c * SITES_PER_CORE : (c + 1) * SITES_PER_CORE]
        nsp = np.zeros((PAD_SITES_CORE, N_PERM, N_NEIGH), dtype=np.int64)
        nsp[:SITES_PER_CORE] = ns
        sites = nsp.reshape(128, SPP, N_PERM, N_NEIGH)  # [p, s, q, k]
        t = (sites >> 1).astype(np.int16)
        par = (sites & 1).astype(np.int8)
        # global col ordering per partition: (s, q) -> s*12+q, split into
        # chunks of GCOLS; position i = c8*128 + p
        arr = (
            t.reshape(128, SPP * N_PERM, N_NEIGH)
            .transpose(1, 2, 0)  # [col, k, p]
            .reshape(N_CHUNKS, GCOLS, N_NEIGH, 128)
            .transpose(0, 2, 1, 3)  # [chunk, k, c8, p]
            .reshape(N_CHUNKS, N_NEIGH, NIDX)
        )
        idxv = np.ascontiguousarray(
            arr.reshape(N_CHUNKS, N_NEIGH, NIDX // 16, 16).transpose(0, 3, 1, 2)
        )
        mskv = np.ascontiguousarray(
            (1 - par)
            .reshape(128, SPP * N_PERM, N_NEIGH)
            .transpose(1, 2, 0)  # [col, k, p]
            .reshape(N_CHUNKS, GCOLS, N_NEIGH, 128)
            .transpose(0, 3, 2, 1)  # [chunk, p, k, c8]
            .astype(np.int8)
        )
        in_maps.append(
            {
                "x2t": np.ascontiguousarray(x2t[16 * c : 16 * (c + 1)]),
                "rhs2": rhs2,
                "bias": bias,
                "idx": idxv,
                "msk": mskv,
            }
        )
    return in_maps


_NC_CACHE = {}


def _get_nc():
    if "nc" not in _NC_CACHE:
        _NC_CACHE["nc"] = build_nc()
    return _NC_CACHE["nc"]


def _stitch(results):
    full = np.empty((N_SITES, OUT_F), dtype=np.float32)
    for c, r in enumerate(results):
        o = r["out"].astype(np.float32).reshape(PAD_SITES_CORE, OUT_F)[:SITES_PER_CORE]
        full[c * SITES_PER_CORE : (c + 1) * SITES_PER_CORE] = o
    return full


def kernel(X_sites, X_NSs, W, b, _trace=False):
    nc = _get_nc()
    in_maps = _host_prep(X_sites, X_NSs, W, b)
    res = run_bass_kernel_spmd(
        nc, in_maps, core_ids=list(range(N_CORES)), trace=_trace
    )
    full = _stitch(res.results)
    if _trace:
        return full, res
    return full


# revision 9
# speedup vs baseline: 18.9086x; 1.1019x over previous
"""Trainium2 Bass kernel for nn_LCNNConvolution (GNN message passing), v4.

Math:  out[n] = sum_p softplus( gather(X, NS[n,p,:]).flat @ W.T + b ) - 12*ln2
Key transform: W is block-structured over the 8 neighbor slots, so
    x1[n,p,:] = sum_k Y_k[NS[n,p,k]]  with  Y_k = X @ W_k.T  (b baked in slot 7).

Design (vs the 2-bank v1): the Y table stores SITE PAIRS per row - row t
holds, for each slot k, [Y_k[2t], Y_k[2t+1]] contiguously (128 f16 = 256B per
slot). 25088 rows fit int16 gather indices in ONE bank, so the hot loop
issues exactly ONE 256B dma_gather descriptor per (n,p,k) lookup (v1 needed
two banks = 2x descriptors, each half-wasted on a zero-row dummy). num_idxs
is capped at the HW-safe 1024/call. The pair half is chosen on DVE via
copy_predicated with a per-lookup int8 parity mask (free-dim stride-0
broadcast AP; output AP rank-matched via a padded-(k c)-merge). The
pair-interleaved table is produced directly by the phase-1 matmul: lhsT =
pair-interleaved X.T (contraction dim = 2 sites x 64 feats), rhs =
block-diagonal-by-parity weights [128, 1024], so no on-chip transpose or
strided DRAM writes are needed.

Upload minimization (wall time through the axon tunnel is transfer-bound):
X.T is uploaded as a per-core 1/8 shard and AllGathered on-device across the
8 cores; bias is a single row partition_broadcast on-device; the parity mask
is int8; the output is f16. ~2.9 MB/core of inputs per run vs ~33 MB for v1.

Sharding: data-parallel over sites; each of the 8 cores handles 6250 sites
(50 sites per partition, perm-major column order) and computes its own full
pair-interleaved Y table from the AllGathered X (replicated W).
"""

import numpy as np

import concourse.bass as bass
import concourse.bacc as bacc
import concourse.mybir as mybir
import concourse.tile as tile
from concourse.bass_utils import run_bass_kernel_spmd

N_SITES = 50000
NODE_F = 64
N_PERM = 12
N_NEIGH = 8
OUT_F = 64

N_CORES = 8
SITES_PER_CORE = N_SITES // N_CORES            # 6250
SPP = 50                                       # sites per partition (pad 6400)
PAD_SITES_CORE = 128 * SPP                     # 6400

SITES_PAD = 50176                              # 2 * 25088 (table pad)
T_ROWS = SITES_PAD // 2                        # 25088 pair rows
NBLK = T_ROWS // 128                           # 196 phase-1 blocks

GCOLS = 8                                      # gather cols per partition/call
NIDX = 128 * GCOLS                             # 1024
RCOLS = 24                                     # reduce group = 2 sites
N_CHUNKS = SPP * N_PERM // GCOLS               # 75

F32 = mybir.dt.float32
F16 = mybir.dt.float16
I16 = mybir.dt.int16
I8 = mybir.dt.int8


def build_nc():
    nc = bacc.Bacc("TRN2", target_bir_lowering=False, debug=False)

    x2t = nc.dram_tensor("x2t", [16, T_ROWS], F16, kind="ExternalInput").ap()
    rhs2 = nc.dram_tensor("rhs2", [128, 1024], F16, kind="ExternalInput").ap()
    bias = nc.dram_tensor("bias", [1, 1024], F32, kind="ExternalInput").ap()
    idx = nc.dram_tensor(
        "idx", [N_CHUNKS, 16, N_NEIGH, NIDX // 16], I16, kind="ExternalInput"
    ).ap()
    msk = nc.dram_tensor(
        "msk", [N_CHUNKS, 128, N_NEIGH, GCOLS], I8, kind="ExternalInput"
    ).ap()
    out = nc.dram_tensor(
        "out", [128, SPP * OUT_F], F16, kind="ExternalOutput"
    ).ap()

    with tile.TileContext(nc) as tc:
        with (
            tc.tile_pool(name="persist", bufs=1) as persist,
            tc.tile_pool(name="dram", bufs=1, space="DRAM") as dram,
        ):
            ybig = dram.tile([T_ROWS, 1024], F16)
            half_sb = persist.tile([128, 1], F32)
            nc.vector.memset(half_sb[:], 0.5)

            # all-gather the pair-interleaved X.T from per-core shards
            # (Shared-addr-space output = the fast RDH AllGather path)
            x2t_in = dram.tile([16, T_ROWS], F16)
            x2t_full = nc.dram_tensor(
                "x2t_full_sh", [128, T_ROWS], F16, addr_space="Shared"
            ).ap()
            with tc.tile_pool(name="p0", bufs=1) as p0:
                sh_sb = p0.tile([16, T_ROWS], F16)
                nc.sync.dma_start(out=sh_sb[:], in_=x2t[:])
                nc.sync.dma_start(out=x2t_in[:], in_=sh_sb[:])
                nc.gpsimd.collective_compute(
                    "AllGather",
                    mybir.AluOpType.bypass,
                    replica_groups=[list(range(8))],
                    ins=[x2t_in.opt()],
                    outs=[x2t_full],
                )

            # ---------------- phase 1: pair-interleaved Y table
            with (
                tc.tile_pool(name="p1", bufs=1) as p1,
                tc.tile_pool(name="p1y", bufs=3) as p1y,
                tc.tile_pool(name="p1ps", bufs=2, space="PSUM") as p1ps,
            ):
                x2t_sb = p1.tile([128, T_ROWS], F16)
                nc.sync.dma_start(out=x2t_sb[:], in_=x2t_full[:])
                rhs2_sb = p1.tile([128, 1024], F16)
                nc.sync.dma_start(out=rhs2_sb[:], in_=rhs2[:])
                bias_row = p1.tile([1, 1024], F32)
                nc.sync.dma_start(out=bias_row[:], in_=bias[:])
                bias_sb = p1.tile([128, 1024], F32)
                nc.gpsimd.partition_broadcast(
                    out_ap=bias_sb[:], in_ap=bias_row[:]
                )

                for j in range(NBLK):
                    psum = p1ps.tile([128, 1024], F32, space="PSUM", tag="ps")
                    lhsT = x2t_sb[:, j * 128 : (j + 1) * 128]
                    nc.tensor.matmul(
                        out=psum[:, 0:512],
                        lhsT=lhsT,
                        rhs=rhs2_sb[:, 0:512],
                        start=True,
                        stop=True,
                    )
                    nc.tensor.matmul(
                        out=psum[:, 512:1024],
                        lhsT=lhsT,
                        rhs=rhs2_sb[:, 512:1024],
                        start=True,
                        stop=True,
                    )
                    y_sb = p1y.tile([128, 1024], F16, tag="y")
                    nc.vector.tensor_tensor(
                        out=y_sb[:],
                        in0=psum[:],
                        in1=bias_sb[:],
                        op=mybir.AluOpType.add,
                    )
                    nc.sync.dma_start(
                        out=ybig[j * 128 : (j + 1) * 128, :], in_=y_sb[:]
                    )

            # ---------------- phase 2: single-bank pair gather + select
            with (
                tc.tile_pool(name="p2", bufs=2) as p2,
                tc.tile_pool(name="p2s", bufs=2) as p2s,
            ):
                x1 = None
                for j in range(N_CHUNKS):
                    idx_sb = p2.tile([128, N_NEIGH, NIDX // 16], I16, tag="idx")
                    nc.sync.dma_start(
                        out=idx_sb[:],
                        in_=idx[j]
                        .rearrange("(o p) k n -> o p k n", o=1)
                        .to_broadcast([8, 16, N_NEIGH, NIDX // 16]),
                    )
                    m_sb = p2.tile([128, N_NEIGH, GCOLS], I8, tag="m")
                    nc.sync.dma_start(out=m_sb[:], in_=msk[j])

                    g = p2.tile([128, N_NEIGH, GCOLS, 128], F16, tag="g")
                    for k in range(N_NEIGH):
                        nc.gpsimd.dma_gather(
                            out_ap=g[:, k, :, :],
                            in_ap=ybig[:, k * 128 : (k + 1) * 128],
                            idxs_ap=idx_sb[:, k, :],
                            num_idxs=NIDX,
                            num_idxs_reg=NIDX,
                            elem_size=128,
                            elem_step=1024,
                        )
                    # pair-half select (per-chunk sel tile)
                    sel = p2.tile([128, N_NEIGH, GCOLS, 72], F16, tag="sel")
                    sel_out = sel[:, :, :, 0:64].rearrange("p k c f -> p (k c) f")
                    nc.vector.tensor_copy(sel_out, g[:, :, :, 64:128])
                    nc.vector.copy_predicated(
                        sel_out,
                        m_sb[:]
                        .rearrange("p k c -> p (k c)")
                        .rearrange("p (m o) -> p m o", o=1)
                        .to_broadcast([128, N_NEIGH * GCOLS, 64]),
                        g[:, :, :, 0:64],
                    )
                    if j % 3 == 0:
                        x1 = p2s.tile([128, RCOLS, 64], F32, tag="x1")
                    sub = j % 3
                    nc.vector.tensor_reduce(
                        out=x1[:, sub * GCOLS : (sub + 1) * GCOLS, :],
                        in_=sel[:, :, :, 0:64].rearrange("p k c f -> p c f k"),
                        axis=mybir.AxisListType.X,
                        op=mybir.AluOpType.add,
                    )
                    if sub != 2:
                        continue
                    grp = j // 3  # covers sites 2*grp, 2*grp+1 per partition
                    # softplus(x) - ln2 == Ln(0.5*Exp(x) + 0.5)
                    x2 = p2s.tile([128, RCOLS, 64], F32, tag="x2")
                    nc.scalar.activation(
                        out=x2[:], in_=x1[:], func=mybir.ActivationFunctionType.Exp
                    )
                    nc.scalar.activation(
                        out=x2[:],
                        in_=x2[:],
                        func=mybir.ActivationFunctionType.Ln,
                        scale=0.5,
                        bias=half_sb[:],
                    )
                    acc = p2s.tile([128, 2, 64], F16, tag="acc")
                    with nc.allow_low_precision(reason="12-term softplus sum"):
                        nc.vector.tensor_reduce(
                            out=acc[:],
                            in_=x2[:].rearrange("p (s q) f -> p s f q", q=N_PERM),
                            axis=mybir.AxisListType.X,
                            op=mybir.AluOpType.add,
                        )
                    nc.sync.dma_start(
                        out=out[:, grp * 128 : grp * 128 + 128],
                        in_=acc[:].rearrange("p s f -> p (s f)"),
                    )

    nc.compile()
    return nc


def _host_prep(X_sites, X_NSs, W, b):
    X_sites = np.asarray(X_sites, dtype=np.float32)
    X_NSs = np.asarray(X_NSs)
    W = np.asarray(W, dtype=np.float32)
    b = np.asarray(b, dtype=np.float32)

    Xp = np.zeros((SITES_PAD, NODE_F), dtype=np.float16)
    Xp[:N_SITES] = X_sites.astype(np.float16)
    x2t = np.ascontiguousarray(
        Xp.reshape(T_ROWS, 2, NODE_F).transpose(1, 2, 0).reshape(128, T_ROWS)
    )

    Wk = W.reshape(OUT_F, N_NEIGH, NODE_F)  # [o, k, f']
    rhs2 = np.zeros((128, 1024), dtype=np.float16)
    for par in range(2):
        for k in range(N_NEIGH):
            c0 = k * 128 + par * 64
            rhs2[par * 64 : par * 64 + 64, c0 : c0 + 64] = Wk[:, k, :].T.astype(
                np.float16
            )
    bias = np.zeros((1, 1024), dtype=np.float32)
    for par in range(2):
        c0 = 7 * 128 + par * 64
        bias[0, c0 : c0 + 64] = b

    in_maps = []
    for c in range(N_CORES):
        ns = X_NSs[c * SITES_PER_CORE : (c + 1) * SITES_PER_CORE]
        nsp = np.zeros((PAD_SITES_CORE, N_PERM, N_NEIGH), dtype=np.int64)
        nsp[:SITES_PER_CORE] = ns
        sites = nsp.reshape(128, SPP, N_PERM, N_NEIGH)  # [p, s, q, k]
        t = (sites >> 1).astype(np.int16)
        par = (sites & 1).astype(np.int8)
        # global col ordering per partition: (s, q) -> s*12+q, split into
        # chunks of GCOLS; position i = c8*128 + p
        arr = (
            t.reshape(128, SPP * N_PERM, N_NEIGH)
            .transpose(1, 2, 0)  # [col, k, p]
            .reshape(N_CHUNKS, GCOLS, N_NEIGH, 128)
            .transpose(0, 2, 1, 3)  # [chunk, k, c8, p]
            .reshape(N_CHUNKS, N_NEIGH, NIDX)
        )
        idxv = np.ascontiguousarray(
            arr.reshape(N_CHUNKS, N_NEIGH, NIDX // 16, 16).transpose(0, 3, 1, 2)
        )
        mskv = np.ascontiguousarray(
            (1 - par)
            .reshape(128, SPP * N_PERM, N_NEIGH)
            .transpose(1, 2, 0)  # [col, k, p]
            .reshape(N_CHUNKS, GCOLS, N_NEIGH, 128)
            .transpose(0, 3, 2, 1)  # [chunk, p, k, c8]
            .astype(np.int8)
        )
        in_maps.append(
            {
                "x2t": np.ascontiguousarray(x2t[16 * c : 16 * (c + 1)]),
                "rhs2": rhs2,
                "bias": bias,
                "idx": idxv,
                "msk": mskv,
            }
        )
    return in_maps


_NC_CACHE = {}


def _get_nc():
    if "nc" not in _NC_CACHE:
        _NC_CACHE["nc"] = build_nc()
    return _NC_CACHE["nc"]


def _stitch(results):
    full = np.empty((N_SITES, OUT_F), dtype=np.float32)
    for c, r in enumerate(results):
        o = r["out"].astype(np.float32).reshape(PAD_SITES_CORE, OUT_F)[:SITES_PER_CORE]
        full[c * SITES_PER_CORE : (c + 1) * SITES_PER_CORE] = o
    return full


def kernel(X_sites, X_NSs, W, b, _trace=False):
    nc = _get_nc()
    in_maps = _host_prep(X_sites, X_NSs, W, b)
    res = run_bass_kernel_spmd(
        nc, in_maps, core_ids=list(range(N_CORES)), trace=_trace
    )
    full = _stitch(res.results)
    if _trace:
        return full, res
    return full


# revision 11
# speedup vs baseline: 19.7342x; 1.0437x over previous
"""Trainium2 Bass kernel for nn_LCNNConvolution (GNN message passing), v6.

Math:  out[n] = sum_p softplus( gather(X, NS[n,p,:]).flat @ W.T + b ) - 12*ln2
Key transform: W is block-structured over the 8 neighbor slots, so
    x1[n,p,:] = sum_k Y_k[NS[n,p,k]]  with  Y_k = X @ W_k.T  (b baked in slot 7).

Design: the Y table stores SITE PAIRS per row - row t holds, for each slot k,
[Y_k[2t], Y_k[2t+1]] contiguously (128 f16 = 256B per slot), so int16 gather
indices cover all 50000 sites in ONE bank and the hot loop issues exactly ONE
256B dma_gather descriptor per (n,p,k) lookup (num_idxs capped at the HW-safe
1024/call). Pair halves are selected on DVE via copy_predicated with an int8
parity mask (free-dim stride-0 broadcast AP). The pair-interleaved layout is
produced directly by the phase-1 matmul: lhsT = pair-interleaved X.T slice,
rhs = block-diagonal-by-parity weights [128, 1024].

v6: phase-1 is SHARDED across the 8 cores - each core uploads only its 1/8
t-range of the pair-interleaved X.T (0.82 MB), computes 25 matmul blocks of
the Y table, and the full table is assembled with an HBM AllGather into a
Shared-addr-space tensor (the fast RDH path), then copied once to local DRAM
for the gathers. Per-chunk idx/msk loads are hoisted into single upfront
DMAs. This minimizes both per-run upload (~2.9 MB/core) and instruction count
(the dominant per-instruction dispatch overhead on this runtime).

Sharding: data-parallel over sites for phase 2 (6250 sites/core, 50 per
partition, perm-major column order); data-parallel over table rows for
phase 1; weights replicated.
"""

import numpy as np

import concourse.bass as bass
import concourse.bacc as bacc
import concourse.mybir as mybir
import concourse.tile as tile
from concourse.bass_utils import run_bass_kernel_spmd

N_SITES = 50000
NODE_F = 64
N_PERM = 12
N_NEIGH = 8
OUT_F = 64

N_CORES = 8
SITES_PER_CORE = N_SITES // N_CORES            # 6250
SPP = 50                                       # sites per partition (pad 6400)
PAD_SITES_CORE = 128 * SPP                     # 6400

T_ROWS = 25088                                 # real pair rows (2*25088 sites)
TPC = 3200                                     # table rows per core (25 blocks)
T_FULL = TPC * N_CORES                         # 25600 (padded table rows)
NBLK = TPC // 128                              # 25 phase-1 blocks per core

GCOLS = 8                                      # gather cols per partition/call
NIDX = 128 * GCOLS                             # 1024
RCOLS = 24                                     # reduce group = 2 sites
N_CHUNKS = SPP * N_PERM // GCOLS               # 75

F32 = mybir.dt.float32
F16 = mybir.dt.float16
I16 = mybir.dt.int16
I8 = mybir.dt.int8


def build_nc():
    nc = bacc.Bacc("TRN2", target_bir_lowering=False, debug=False)

    x2t = nc.dram_tensor("x2t", [128, TPC], F16, kind="ExternalInput").ap()
    rhs2 = nc.dram_tensor("rhs2", [128, 1024], F16, kind="ExternalInput").ap()
    bias = nc.dram_tensor("bias", [1, 1024], F32, kind="ExternalInput").ap()
    idx = nc.dram_tensor(
        "idx", [N_CHUNKS, 16, N_NEIGH, NIDX // 16], I16, kind="ExternalInput"
    ).ap()
    msk = nc.dram_tensor(
        "msk", [N_CHUNKS, 128, N_NEIGH, GCOLS], I8, kind="ExternalInput"
    ).ap()
    out = nc.dram_tensor(
        "out", [128, SPP * OUT_F], F16, kind="ExternalOutput"
    ).ap()

    with tile.TileContext(nc) as tc:
        with (
            tc.tile_pool(name="persist", bufs=1) as persist,
            tc.tile_pool(name="dram", bufs=1, space="DRAM") as dram,
        ):
            ybig = dram.tile([T_FULL, 1024], F16)
            y_part = dram.tile([TPC, 1024], F16)
            y_sh = nc.dram_tensor(
                "y_gathered_sh", [T_FULL, 1024], F16, addr_space="Shared"
            ).ap()
            half_sb = persist.tile([128, 1], F32)
            nc.vector.memset(half_sb[:], 0.5)

            # ---------------- phase 1 (sharded): this core's 1/8 of the
            # pair-interleaved Y table, then AllGather + copy to local DRAM
            with (
                tc.tile_pool(name="p1", bufs=1) as p1,
                tc.tile_pool(name="p1y", bufs=3) as p1y,
                tc.tile_pool(name="p1ps", bufs=2, space="PSUM") as p1ps,
            ):
                x2t_sb = p1.tile([128, TPC], F16)
                nc.sync.dma_start(out=x2t_sb[:], in_=x2t[:])
                rhs2_sb = p1.tile([128, 1024], F16)
                nc.sync.dma_start(out=rhs2_sb[:], in_=rhs2[:])
                bias_row = p1.tile([1, 1024], F32)
                nc.sync.dma_start(out=bias_row[:], in_=bias[:])
                bias_sb = p1.tile([128, 1024], F32)
                nc.gpsimd.partition_broadcast(
                    out_ap=bias_sb[:], in_ap=bias_row[:]
                )

                for j in range(NBLK):
                    psum = p1ps.tile([128, 1024], F32, space="PSUM", tag="ps")
                    lhsT = x2t_sb[:, j * 128 : (j + 1) * 128]
                    nc.tensor.matmul(
                        out=psum[:, 0:512],
                        lhsT=lhsT,
                        rhs=rhs2_sb[:, 0:512],
                        start=True,
                        stop=True,
                    )
                    nc.tensor.matmul(
                        out=psum[:, 512:1024],
                        lhsT=lhsT,
                        rhs=rhs2_sb[:, 512:1024],
                        start=True,
                        stop=True,
                    )
                    y_sb = p1y.tile([128, 1024], F16, tag="y")
                    nc.vector.tensor_tensor(
                        out=y_sb[:],
                        in0=psum[:],
                        in1=bias_sb[:],
                        op=mybir.AluOpType.add,
                    )
                    nc.sync.dma_start(
                        out=y_part[j * 128 : (j + 1) * 128, :], in_=y_sb[:]
                    )

                nc.gpsimd.collective_compute(
                    "AllGather",
                    mybir.AluOpType.bypass,
                    replica_groups=[list(range(N_CORES))],
                    ins=[y_part.opt()],
                    outs=[y_sh],
                )
                nc.sync.dma_start(out=ybig[:], in_=y_sh[:])

            # ---------------- phase 2: single-bank pair gather + select
            with (
                tc.tile_pool(name="p2", bufs=2) as p2,
                tc.tile_pool(name="p2c", bufs=1) as p2c,
                tc.tile_pool(name="p2s", bufs=2) as p2s,
            ):
                # all idx (16-wrap replicated to 128 partitions) and all msk
                # in two upfront DMAs
                idx_all = p2c.tile(
                    [128, N_CHUNKS, N_NEIGH, NIDX // 16], I16
                )
                idx_t = idx[:].rearrange("c p k n -> p c k n")
                for a in range(8):
                    nc.sync.dma_start(
                        out=idx_all[16 * a : 16 * (a + 1)], in_=idx_t
                    )
                msk_all = p2c.tile([128, N_CHUNKS, N_NEIGH, GCOLS], I8)
                nc.sync.dma_start(
                    out=msk_all[:],
                    in_=msk[:].rearrange("c p k g -> p c k g"),
                )

                x1 = None
                for j in range(N_CHUNKS):
                    g = p2.tile([128, N_NEIGH, GCOLS, 128], F16, tag="g")
                    for k in range(N_NEIGH):
                        nc.gpsimd.dma_gather(
                            out_ap=g[:, k, :, :],
                            in_ap=ybig[:, k * 128 : (k + 1) * 128],
                            idxs_ap=idx_all[:, j, k, :],
                            num_idxs=NIDX,
                            num_idxs_reg=NIDX,
                            elem_size=128,
                            elem_step=1024,
                        )
                    # pair-half select (per-chunk sel tile)
                    sel = p2.tile([128, N_NEIGH, GCOLS, 72], F16, tag="sel")
                    sel_out = sel[:, :, :, 0:64].rearrange("p k c f -> p (k c) f")
                    nc.vector.tensor_copy(sel_out, g[:, :, :, 64:128])
                    nc.vector.copy_predicated(
                        sel_out,
                        msk_all[:, j]
                        .rearrange("p k c -> p (k c)")
                        .rearrange("p (m o) -> p m o", o=1)
                        .to_broadcast([128, N_NEIGH * GCOLS, 64]),
                        g[:, :, :, 0:64],
                    )
                    if j % 3 == 0:
                        x1 = p2s.tile([128, RCOLS, 64], F32, tag="x1")
                    sub = j % 3
                    nc.vector.tensor_reduce(
                        out=x1[:, sub * GCOLS : (sub + 1) * GCOLS, :],
                        in_=sel[:, :, :, 0:64].rearrange("p k c f -> p c f k"),
                        axis=mybir.AxisListType.X,
                        op=mybir.AluOpType.add,
                    )
                    if sub != 2:
                        continue
                    grp = j // 3  # covers sites 2*grp, 2*grp+1 per partition
                    # softplus(x) - ln2 == Ln(0.5*Exp(x) + 0.5)
                    x2 = p2s.tile([128, RCOLS, 64], F32, tag="x2")
                    nc.scalar.activation(
                        out=x2[:], in_=x1[:], func=mybir.ActivationFunctionType.Exp
                    )
                    nc.scalar.activation(
                        out=x2[:],
                        in_=x2[:],
                        func=mybir.ActivationFunctionType.Ln,
                        scale=0.5,
                        bias=half_sb[:],
                    )
                    acc = p2s.tile([128, 2, 64], F16, tag="acc")
                    with nc.allow_low_precision(reason="12-term softplus sum"):
                        nc.vector.tensor_reduce(
                            out=acc[:],
                            in_=x2[:].rearrange("p (s q) f -> p s f q", q=N_PERM),
                            axis=mybir.AxisListType.X,
                            op=mybir.AluOpType.add,
                        )
                    nc.sync.dma_start(
                        out=out[:, grp * 128 : grp * 128 + 128],
                        in_=acc[:].rearrange("p s f -> p (s f)"),
                    )

    nc.compile()
    return nc


def _host_prep(X_sites, X_NSs, W, b):
    X_sites = np.asarray(X_sites, dtype=np.float32)
    X_NSs = np.asarray(X_NSs)
    W = np.asarray(W, dtype=np.float32)
    b = np.asarray(b, dtype=np.float32)

    # pair-interleaved X.T over the padded table: col t = pair (2t, 2t+1)
    Xp = np.zeros((2 * T_FULL, NODE_F), dtype=np.float16)
    Xp[:N_SITES] = X_sites.astype(np.float16)
    x2t_full = np.ascontiguousarray(
        Xp.reshape(T_FULL, 2, NODE_F).transpose(1, 2, 0).reshape(128, T_FULL)
    )

    Wk = W.reshape(OUT_F, N_NEIGH, NODE_F)  # [o, k, f']
    rhs2 = np.zeros((128, 1024), dtype=np.float16)
    for par in range(2):
        for k in range(N_NEIGH):
            c0 = k * 128 + par * 64
            rhs2[par * 64 : par * 64 + 64, c0 : c0 + 64] = Wk[:, k, :].T.astype(
                np.float16
            )
    bias = np.zeros((1, 1024), dtype=np.float32)
    for par in range(2):
        c0 = 7 * 128 + par * 64
        bias[0, c0 : c0 + 64] = b

    in_maps = []
    for c in range(N_CORES):
        ns = X_NSs[c * SITES_PER_CORE : (c + 1) * SITES_PER_CORE]
        nsp = np.zeros((PAD_SITES_CORE, N_PERM, N_NEIGH), dtype=np.int64)
        nsp[:SITES_PER_CORE] = ns
        sites = nsp.reshape(128, SPP, N_PERM, N_NEIGH)  # [p, s, q, k]
        t = (sites >> 1).astype(np.int16)
        par = (sites & 1).astype(np.int8)
        arr = (
            t.reshape(128, SPP * N_PERM, N_NEIGH)
            .transpose(1, 2, 0)  # [col, k, p]
            .reshape(N_CHUNKS, GCOLS, N_NEIGH, 128)
            .transpose(0, 2, 1, 3)  # [chunk, k, c8, p]
            .reshape(N_CHUNKS, N_NEIGH, NIDX)
        )
        idxv = np.ascontiguousarray(
            arr.reshape(N_CHUNKS, N_NEIGH, NIDX // 16, 16).transpose(0, 3, 1, 2)
        )
        mskv = np.ascontiguousarray(
            (1 - par)
            .reshape(128, SPP * N_PERM, N_NEIGH)
            .transpose(1, 2, 0)  # [col, k, p]
            .reshape(N_CHUNKS, GCOLS, N_NEIGH, 128)
            .transpose(0, 3, 2, 1)  # [chunk, p, k, c8]
            .astype(np.int8)
        )
        in_maps.append(
            {
                "x2t": np.ascontiguousarray(x2t_full[:, TPC * c : TPC * (c + 1)]),
                "rhs2": rhs2,
                "bias": bias,
                "idx": idxv,
                "msk": mskv,
            }
        )
    return in_maps


_NC_CACHE = {}


def _get_nc():
    if "nc" not in _NC_CACHE:
        _NC_CACHE["nc"] = build_nc()
    return _NC_CACHE["nc"]


def _stitch(results):
    full = np.empty((N_SITES, OUT_F), dtype=np.float32)
    for c, r in enumerate(results):
        o = r["out"].astype(np.float32).reshape(PAD_SITES_CORE, OUT_F)[
            :SITES_PER_CORE
        ]
        full[c * SITES_PER_CORE : (c + 1) * SITES_PER_CORE] = o
    return full


def kernel(X_sites, X_NSs, W, b, _trace=False):
    nc = _get_nc()
    in_maps = _host_prep(X_sites, X_NSs, W, b)
    res = run_bass_kernel_spmd(
        nc, in_maps, core_ids=list(range(N_CORES)), trace=_trace
    )
    full = _stitch(res.results)
    if _trace:
        return full, res
    return full
